# revision 31
# baseline (speedup 1.0000x reference)
"""GQA causal self-attention (B=2, T=2048, C=2048, 16 Q heads / 4 KV heads,
head_dim=128) on 8 TRN2 NeuronCores.

Sharding: core = (batch b, kv-group g) for b in {0,1}, g in {0..3}.
Each core computes its batch's 4 Q heads that share KV head g, plus the
partial out-projection over those heads' rows of W_out. Host sums the 4
partials per batch and adds b_out.

v2 changes vs baseline:
  - Q/K projection in fp8-e4m3 with perf_mode=DoubleRow (2 contraction
    chunks per matmul): ~1.7x fewer PE cycles on 5/6 of the QKV GEMM.
    x and W_qk pre-scaled by powers of 2 (folded into the exp scale).
  - RoPE pair-swap without the PE swap-matmul: the q/k head dims are
    host-permuted so each (even, odd) pair sits 16 partitions apart within
    a 32-partition quadrant; the swap is one DVE stream_shuffle op.
  - Denominator: no per-jt ones-matmuls. P tiles are accumulated in SBUF
    (DVE adds, bf16); one M=1 ones-matmul per (head, s) computes the
    denominator row, at PSUM partition 0 (partition-32 PSUM rows are not
    readable: DVE custom ops silently drop the AP base partition).
  - 1/den via reciprocal_approx_fast (single DVE op, 18-bit accurate).
  - exp outputs bf16 (2x ACT rate).
  - PSUM budget = exactly 8 banks via shared pool tags across phases.
"""

import sys

if "/opt/trn_rl_repo" not in sys.path:
    sys.path.insert(0, "/opt/trn_rl_repo")

import numpy as np
import ml_dtypes

BF16 = ml_dtypes.bfloat16
E4M3 = ml_dtypes.float8_e4m3

B = 2
T = 2048
C = 2048
NH = 16
NKV = 4
D = 128
GQ = NH // NKV  # 4 q heads per kv head
N_CORES = 8
CC = C // 128  # 16 contraction chunks
TS = T // 512  # 4 t-slices
TT = T // 128  # 16 t-tiles
NQK = GQ + 1  # q0..q3, k

S_X = 16.0  # x pre-scale for fp8
S_W = 512.0  # W_qk pre-scale for fp8
SC_SCALE = 1.0 / (128.0 * (S_X * S_W) ** 2)  # exp scale: 1/128 + undo fp8 scales

_CACHED = {}


def _build_bass(reps=1, dbg=False):
    import concourse.bass as bass
    import concourse.bacc as bacc
    import concourse.tile as tile
    import concourse.mybir as mybir

    bf = mybir.dt.bfloat16
    f32 = mybir.dt.float32
    f8 = mybir.dt.float8e4

    nc = bacc.Bacc(None, target_bir_lowering=False)

    xt_d = nc.dram_tensor("xt", [128, CC, T], bf, kind="ExternalInput")
    x8_d = nc.dram_tensor("x8", [128, CC, T], f8, kind="ExternalInput")
    wqk_d = nc.dram_tensor("wqk", [128, CC, NQK * 128], f8, kind="ExternalInput")
    wv_d = nc.dram_tensor("wv", [128, CC, 128], bf, kind="ExternalInput")
    bqk_d = nc.dram_tensor("bqk", [128, NQK], f32, kind="ExternalInput")
    bv_d = nc.dram_tensor("bv", [128, 1], f32, kind="ExternalInput")
    cos_d = nc.dram_tensor("cosT", [128, T], bf, kind="ExternalInput")
    sin_d = nc.dram_tensor("sinT", [128, T], bf, kind="ExternalInput")
    iden_d = nc.dram_tensor("idn", [128, 128], bf, kind="ExternalInput")
    wout_d = nc.dram_tensor("wout", [128, GQ, C], bf, kind="ExternalInput")
    out_d = nc.dram_tensor("out", [T, C], f32, kind="ExternalOutput")
    dbg_d = None
    if dbg:
        dbg_d = {
            "qk": nc.dram_tensor("dqk", [128, NQK, T], bf, kind="ExternalOutput"),
            "v": nc.dram_tensor("dv", [128, TT, 128], bf, kind="ExternalOutput"),
            "y": nc.dram_tensor("dy", [128, GQ, T], bf, kind="ExternalOutput"),
        }

    with tile.TileContext(nc) as tc:
        with (
            tc.tile_pool(name="persist", bufs=1) as pers,
            tc.tile_pool(name="xtb", bufs=2) as xtp,
            tc.tile_pool(name="xt8", bufs=2) as x8p,
            tc.tile_pool(name="stage", bufs=4) as stg,
            tc.tile_pool(name="ptile", bufs=6) as ptp,
            tc.tile_pool(name="small", bufs=4) as smp,
            tc.tile_pool(name="osb", bufs=2) as osp,
            tc.tile_pool(name="ps", bufs=1, space="PSUM") as psp,
        ):
            import contextlib
            loop_cm = tc.For_i(0, reps, 1) if reps > 1 else contextlib.nullcontext()
            with loop_cm:
                _body(nc, tc, mybir, bf, f32, f8,
                      pers, xtp, x8p, stg, ptp, smp, osp, psp,
                      xt_d, x8_d, wqk_d, wv_d, bqk_d, bv_d, cos_d, sin_d,
                      iden_d, wout_d, out_d, dbg_d)
    nc.compile()
    return nc


def _body(nc, tc, mybir, bf, f32, f8,
          pers, xtp, x8p, stg, ptp, smp, osp, psp,
          xt_d, x8_d, wqk_d, wv_d, bqk_d, bv_d, cos_d, sin_d,
          iden_d, wout_d, out_d, dbg_d=None):
    Exp = mybir.ActivationFunctionType.Exp
    DR = mybir.MatmulPerfMode.DoubleRow
    mul = mybir.AluOpType.mult
    add = mybir.AluOpType.add

    # ---- persistent loads (ordered so first QKV matmul can start early) ----
    wqk_sb = pers.tile([128, CC, NQK * 128], f8)
    nc.sync.dma_start(wqk_sb[:], wqk_d[:])
    x8_0 = x8p.tile([128, CC, 512], f8, tag="x8")
    nc.sync.dma_start(x8_0[:, 0:4, :], x8_d[:, 0:4, 0:512])
    nc.sync.dma_start(x8_0[:, 4:16, :], x8_d[:, 4:16, 0:512])
    bqk_sb = pers.tile([128, NQK], f32)
    nc.sync.dma_start(bqk_sb[:], bqk_d[:])
    bv_sb = pers.tile([128, 1], f32)
    nc.sync.dma_start(bv_sb[:], bv_d[:])
    cos_sb = pers.tile([128, T], bf)
    nc.sync.dma_start(cos_sb[:], cos_d[:])
    sin_sb = pers.tile([128, T], bf)
    nc.sync.dma_start(sin_sb[:], sin_d[:])
    xt0 = xtp.tile([128, CC, 512], bf, tag="xt")
    for half in range(2):
        nc.sync.dma_start(xt0[:, half * 8:(half + 1) * 8, :],
                          xt_d[:, half * 8:(half + 1) * 8, 0:512])
    wv_sb = pers.tile([128, CC, 128], bf)
    nc.sync.dma_start(wv_sb[:], wv_d[:])
    iden_sb = pers.tile([128, 128], bf)
    nc.sync.dma_start(iden_sb[:], iden_d[:])
    wout_sb = pers.tile([128, GQ, C], bf)
    nc.sync.dma_start(wout_sb[:], wout_d[:])
    ones_sb = pers.tile([128, 1], bf)
    nc.vector.memset(ones_sb[:], 1.0)

    # persistent activations
    qk_sb = pers.tile([128, NQK, T], bf)  # rotated q0..q3, k (fp8-scaled)
    v_sb = pers.tile([128, TT, 128], bf)  # v in [t-part, d] tiles
    y_sb = pers.tile([128, GQ, T], bf)    # normalized y^T per head

    # ---- phase 1: QKV + RoPE + v transpose ----
    for ts in range(TS):
        tsl = slice(ts * 512, (ts + 1) * 512)
        if ts == 0:
            x8 = x8_0
            xt = xt0
        else:
            x8 = x8p.tile([128, CC, 512], f8, tag="x8")
            nc.sync.dma_start(x8[:], x8_d[:, :, tsl])
            xt = xtp.tile([128, CC, 512], bf, tag="xt")
            nc.sync.dma_start(xt[:], xt_d[:, :, tsl])
        for f in range(NQK):
            ps = psp.tile([128, 512], f32, tag="mm", bufs=3)
            for c in range(CC // 2):
                nc.tensor.matmul(
                    ps[:],
                    wqk_sb[:, 2 * c : 2 * c + 2, f * 128 : (f + 1) * 128],
                    x8[:, 2 * c : 2 * c + 2, :],
                    start=(c == 0),
                    stop=(c == CC // 2 - 1),
                    perf_mode=DR,
                )
            raw = stg.tile([128, 512], bf, tag="raw")
            nc.vector.tensor_tensor(
                raw[:], ps[:], bqk_sb[:, f : f + 1].to_broadcast((128, 512)), add
            )
            # rope: pair partners sit 16 apart within each 32-partition
            # quadrant (host perm), so the swap is one stream_shuffle.
            # sinT carries the sign: rows r%32<16 hold -sin, else +sin.
            tmp = stg.tile([128, 512], bf, tag="ropetmp")
            nc.vector.stream_shuffle(
                tmp[:], raw[:], mask=list(range(16, 32)) + list(range(16))
            )
            nc.vector.tensor_tensor(tmp[:], tmp[:], sin_sb[:, tsl], mul)
            nc.vector.tensor_tensor(qk_sb[:, f, tsl], raw[:], cos_sb[:, tsl], mul)
            nc.vector.tensor_tensor(qk_sb[:, f, tsl], qk_sb[:, f, tsl], tmp[:], add)
        # v: bf16 matmul, then transpose [d, t] -> [t, d] via PE
        ps = psp.tile([128, 512], f32, tag="mm", bufs=3)
        for cc in range(CC):
            nc.tensor.matmul(
                ps[:], wv_sb[:, cc, :], xt[:, cc, :],
                start=(cc == 0), stop=(cc == CC - 1),
            )
        raw = stg.tile([128, 512], bf, tag="raw")
        nc.vector.tensor_tensor(
            raw[:], ps[:], bv_sb[:, 0:1].to_broadcast((128, 512)), add
        )
        pst = psp.tile([128, 512], bf, tag="tr", bufs=1)
        for k in range(4):
            nc.tensor.transpose(
                pst[:, k * 128 : (k + 1) * 128],
                raw[:, k * 128 : (k + 1) * 128],
                iden_sb[:],
            )
        nc.vector.tensor_copy(v_sb[:, ts * 4 : ts * 4 + 4, :], pst[:])

    # ---- phase 2: attention, head pairs; phase 3: out-proj per s ----
    for s in range(TS):
        isl = slice(s * 512, (s + 1) * 512)
        njt = 4 * (s + 1)
        for pair in range(2):
            psy = [
                psp.tile([128, 512], f32, tag="acc", bufs=2, name=f"psy{i}")
                for i in range(2)
            ]
            pacc = [
                stg.tile([128, 512], bf, tag="pacc", bufs=4, name=f"pacc{i}")
                for i in range(2)
            ]
            for jt in range(njt):
                off = max(0, 128 * jt - 512 * s)
                for i in range(2):
                    h = 2 * pair + i
                    pss = psp.tile([128, 512], f32, tag="mm", bufs=3, name="pss")
                    nc.tensor.matmul(
                        pss[:, off:512],
                        qk_sb[:, GQ, jt * 128 : (jt + 1) * 128],
                        qk_sb[:, h, s * 512 + off : (s + 1) * 512],
                        start=True,
                        stop=True,
                    )
                    P = ptp.tile([128, 512], bf, tag="P")
                    nc.scalar.activation(
                        P[:, off:512], pss[:, off:512], Exp, scale=SC_SCALE
                    )
                    if jt >= 4 * s:
                        # triangular block: keep where p <= col (rel. to off)
                        nc.gpsimd.affine_select(
                            out=P[:, off : off + 128],
                            in_=P[:, off : off + 128],
                            pattern=[[1, 128]],
                            compare_op=mybir.AluOpType.is_ge,
                            fill=0.0,
                            base=0,
                            channel_multiplier=-1,
                        )
                    nc.tensor.matmul(
                        psy[i][:, off:512],
                        v_sb[:, jt, :],
                        P[:, off:512],
                        start=(jt == 0),
                        stop=(jt == njt - 1),
                    )
                    if jt == 0:
                        nc.vector.tensor_copy(pacc[i][:], P[:])
                    else:
                        nc.vector.tensor_tensor(
                            pacc[i][:, off:512], pacc[i][:, off:512],
                            P[:, off:512], add,
                        )
            for i in range(2):
                h = 2 * pair + i
                psd = psp.tile([128, 512], f32, tag="mm", bufs=3, name="psd")
                nc.tensor.matmul(
                    psd[0:1, :], ones_sb[:], pacc[i][:], start=True, stop=True
                )
                rden = smp.tile([1, 512], f32, tag="rden")
                nc.vector.reciprocal_approx_fast(out=rden[:], in_=psd[0:1, :])
                rdb = smp.tile([128, 512], f32, tag="rdb")
                nc.gpsimd.partition_broadcast(rdb[:], rden[:])
                nc.vector.tensor_tensor(y_sb[:, h, isl], psy[i][:], rdb[:], mul)

        for tt in range(4 * s, 4 * s + 4):
            o_sb = osp.tile([128, C], f32, tag="osb")
            for es in range(4):
                pso = psp.tile([128, 512], f32, tag="pso", bufs=2, name="pso")
                for h in range(GQ):
                    nc.tensor.matmul(
                        pso[:],
                        y_sb[:, h, tt * 128 : (tt + 1) * 128],
                        wout_sb[:, h, es * 512 : (es + 1) * 512],
                        start=(h == 0),
                        stop=(h == GQ - 1),
                    )
                if es % 2 == 0:
                    nc.vector.tensor_copy(o_sb[:, es * 512 : (es + 1) * 512], pso[:])
                else:
                    nc.scalar.copy(o_sb[:, es * 512 : (es + 1) * 512], pso[:])
            nc.sync.dma_start(out_d[tt * 128 : (tt + 1) * 128, :], o_sb[:])

    if dbg_d is not None:
        nc.sync.dma_start(dbg_d["qk"][:], qk_sb[:])
        nc.sync.dma_start(dbg_d["v"][:], v_sb[:])
        nc.sync.dma_start(dbg_d["y"][:], y_sb[:])


_PERM = None


def _perm128():
    """New position p (b=p//32, r=p%32) holds original dim 2i + (r>=16),
    where i = 16*b + r%16 — pair partners 16 apart within a quadrant."""
    global _PERM
    if _PERM is None:
        p = np.arange(128)
        b, r = p // 32, p % 32
        i = 16 * b + (r % 16)
        _PERM = 2 * i + (r >= 16)
    return _PERM


def _host_prep(x, rope_cache, W_qkv, b_qkv, W_out):
    """Build the 8 per-core input dicts."""
    q_dim = NH * D  # 2048
    kv_dim = NKV * D  # 512
    perm = _perm128()

    # rope tables in the permuted layout: row p belongs to pair i=16*(p//32)
    # + p%16; sin sign negative on the even-slot half (r%32 < 16)
    sin = rope_cache[:, 0::2].astype(np.float32)  # [T, 64]
    cos = rope_cache[:, 1::2].astype(np.float32)
    cosT = np.empty((128, T), np.float32)
    sinT = np.empty((128, T), np.float32)
    p = np.arange(128)
    bq, r = p // 32, p % 32
    pair = 16 * bq + (r % 16)
    cosT[:] = cos.T[pair]
    sinT[:] = np.where((r < 16)[:, None], -sin.T[pair], sin.T[pair])
    cosT = cosT.astype(BF16)
    sinT = sinT.astype(BF16)

    iden = np.eye(128, dtype=BF16)

    in_maps = []
    for b in range(B):
        xb = x[b]  # [T, C]
        xT = np.ascontiguousarray(xb.T.astype(BF16))  # [C, T]
        xT = xT.reshape(CC, 128, T).transpose(1, 0, 2)  # [128, CC, T]
        xT = np.ascontiguousarray(xT)
        x8 = np.ascontiguousarray(
            (xb.T.astype(np.float32) * S_X).astype(E4M3)
            .reshape(CC, 128, T).transpose(1, 0, 2)
        )  # [128, CC, T] fp8
        for g in range(NKV):
            # q/k columns, with per-head even/odd d-permutation
            qk_cols = []
            for f in range(GQ):
                base = (4 * g + f) * D
                qk_cols.append(base + perm)
            qk_cols.append(q_dim + g * D + perm)  # k head
            qk_cols = np.concatenate(qk_cols)
            wqk = np.ascontiguousarray(
                (W_qkv[:, qk_cols].astype(np.float32) * S_W).astype(E4M3)
                .reshape(CC, 128, NQK * 128).transpose(1, 0, 2)
            )  # [128, CC, 640]
            bqk = np.ascontiguousarray(
                (b_qkv[qk_cols].astype(np.float32) * (S_X * S_W))
                .reshape(NQK, 128).T
            )  # [128, NQK]
            v_cols = np.arange(q_dim + kv_dim + g * D, q_dim + kv_dim + (g + 1) * D)
            wv = np.ascontiguousarray(
                W_qkv[:, v_cols].astype(BF16)
                .reshape(CC, 128, 128).transpose(1, 0, 2)
            )  # [128, CC, 128]
            bv = np.ascontiguousarray(
                b_qkv[v_cols].astype(np.float32).reshape(128, 1)
            )
            wo = W_out[4 * g * D : (4 * g + 4) * D, :].astype(BF16)  # [512, C]
            wo = np.ascontiguousarray(
                wo.reshape(GQ, 128, C).transpose(1, 0, 2)
            )  # [128, GQ, C]
            in_maps.append(
                {
                    "xt": xT,
                    "x8": x8,
                    "wqk": wqk,
                    "wv": wv,
                    "bqk": bqk,
                    "bv": bv,
                    "cosT": cosT,
                    "sinT": sinT,
                    "idn": iden,
                    "wout": wo,
                }
            )
    return in_maps


def kernel(x, rope_cache, W_qkv, b_qkv, W_out, b_out, _trace=False):
    from concourse.bass_utils import run_bass_kernel_spmd

    if "nc" not in _CACHED:
        _CACHED["nc"] = _build_bass()
    nc = _CACHED["nc"]

    in_maps = _host_prep(
        np.asarray(x), np.asarray(rope_cache), np.asarray(W_qkv),
        np.asarray(b_qkv), np.asarray(W_out),
    )
    res = run_bass_kernel_spmd(nc, in_maps, core_ids=list(range(N_CORES)), trace=_trace)
    _CACHED["last_result"] = res

    out = np.zeros((B, T, C), np.float32)
    for b in range(B):
        acc = res.results[b * NKV]["out"].astype(np.float32)
        for g in range(1, NKV):
            acc = acc + res.results[b * NKV + g]["out"]
        out[b] = acc + np.asarray(b_out)[None, :]
    return out
